# revision 10
# baseline (speedup 1.0000x reference)
"""Trainium2 Bass kernel for nn_AdvantageNetwork (gnn_message_passing).

Math (per batch b of B=4096, N=32 agents, d_in=256, D=256):
  x = concat(states, actions)                       [B,N,256]
  K = x Wk^T ; Q = x Wq^T ; V = x Wv^T              [B,N,256]
  score[b,i,j] = <Q[b,j], K[b,i]> / 16
  w = softmax_j(score)                              [B,N,N]
  weighted = w V / N                                [B,N,256]
  h = leaky_relu(weighted W1^T, 0.01)               [B,N,64]
  adv = h W2^T                                      [B,N,16]
  returns (adv, w[..., None])

Kernel algebra:
  score = x A x^T / 16 with A = Wk^T Wq     (skips separate K,Q projections)
  V' = x (W1 Wv)^T / 32                     (fuses V proj, W1 matmul and /N)
  leaky_relu(y) = relu(y) - 0.01 relu(-y)   (both Relus live in the exp ACT
      table set -> no act-table reloads; subtraction folds into the adv
      matmul as a second accumulate against -W2^T)

allpairs variant: score is computed all-pairs per 128-row subtile
  (4 batches), a block-diagonal additive -1e30 mask zeroes cross-batch
  entries under exp, and the PE transpose of the normalized exp matrix IS
  the block-diagonal w^T needed by the weighted matmul (no data shuffling).

Sharding: data-parallel over batch across 8 NeuronCores (512 batches/core).
"""
import numpy as np
from contextlib import ExitStack

import concourse.bass as bass
import concourse.tile as tile
from concourse import bacc, mybir
from concourse.masks import make_identity

N_CORES = 8
B, N, OBS, ACT = 4096, 32, 240, 16
D = 256
BS = B // N_CORES            # 512 batches per core
M = BS * N                   # 16384 rows per core
MT = 512                     # rows per m-tile
NMT = M // MT                # 32 m-tiles
f32 = mybir.dt.float32
f32r = mybir.dt.float32r
AF = mybir.ActivationFunctionType
ALU = mybir.AluOpType


def build_kernel(reps: int = 1, allpairs: bool = True):
    nc = bacc.Bacc("TRN2", target_bir_lowering=False, debug=False,
                   num_devices=N_CORES)
    xin = nc.dram_tensor("xin", [M, D], f32, kind="ExternalInput").ap()
    Wk = nc.dram_tensor("Wk", [D, D], f32, kind="ExternalInput").ap()
    Wq = nc.dram_tensor("Wq", [D, D], f32, kind="ExternalInput").ap()
    Wv = nc.dram_tensor("Wv", [D, D], f32, kind="ExternalInput").ap()
    W1 = nc.dram_tensor("W1", [64, D], f32, kind="ExternalInput").ap()
    W2 = nc.dram_tensor("W2", [16, 64], f32, kind="ExternalInput").ap()
    maskin = nc.dram_tensor("mask", [128, 128], f32, kind="ExternalInput").ap()
    # fused output: cols 0:32 = softmax w, cols 32:48 = adv
    wadv = nc.dram_tensor("wadv", [M, 48], f32, kind="ExternalOutput").ap()

    with tile.TileContext(nc) as tc, ExitStack() as ctx:
        consts = ctx.enter_context(tc.tile_pool(name="consts", bufs=1))

        ident = consts.tile([128, 128], f32)
        make_identity(nc, ident)
        mask = consts.tile([128, 128], f32)
        nc.sync.dma_start(out=mask, in_=maskin)

        # ---------------- weight setup (once) ----------------
        A_sb = consts.tile([128, 2, 256], f32r)   # A = Wk^T Wq, din on part
        Wv1T = consts.tile([128, 2, 64], f32r)    # (W1 Wv)^T / 32, din on part
        W2T = consts.tile([64, 16], f32)          # W2^T
        W2Tn = consts.tile([64, 16], f32)         # -W2^T

        with tc.tile_pool(name="setup_sb", bufs=1) as ssb, \
             tc.tile_pool(name="setup_ps", bufs=1, space="PSUM") as sps:
            wk_sb = ssb.tile([128, 2, 256], f32)   # [k part, k blk, din]
            wq_sb = ssb.tile([128, 2, 256], f32)
            wv_sb = ssb.tile([128, 2, 256], f32)
            w1_sb = ssb.tile([64, 256], f32)
            w2_sb = ssb.tile([16, 64], f32)
            nc.sync.dma_start(out=wk_sb, in_=Wk.rearrange("(kb k) d -> k kb d", k=128))
            nc.sync.dma_start(out=wq_sb, in_=Wq.rearrange("(kb k) d -> k kb d", k=128))
            nc.sync.dma_start(out=wv_sb, in_=Wv.rearrange("(kb k) d -> k kb d", k=128))
            nc.sync.dma_start(out=w1_sb, in_=W1)
            nc.sync.dma_start(out=w2_sb, in_=W2)

            # A rows block db: psum = sum_kb Wk[:,kb,db-slice].T @ Wq[:,kb,:]
            for db in range(2):
                a_ps = sps.tile([128, 256], f32, tag="a")
                for kb in range(2):
                    nc.tensor.matmul(a_ps, wk_sb[:, kb, 128 * db:128 * db + 128],
                                     wq_sb[:, kb, :],
                                     start=(kb == 0), stop=(kb == 1))
                nc.scalar.copy(A_sb[:, db, :], a_ps)

            # W1T [128, 2(kb), 64] via PE transpose of W1 [64, 256]
            w1t = ssb.tile([128, 2, 64], f32)
            for kb in range(2):
                t_ps = sps.tile([128, 64], f32, tag="t")
                nc.tensor.transpose(t_ps, w1_sb[:, 128 * kb:128 * kb + 128],
                                    ident[0:64, 0:64])
                nc.scalar.copy(w1t[:, kb, :], t_ps)

            # Wv1T din-block db = sum_kb Wv[:,kb,db-slice].T @ W1T[:,kb,:], / N
            for db in range(2):
                v_ps = sps.tile([128, 64], f32, tag="t")
                for kb in range(2):
                    nc.tensor.matmul(v_ps, wv_sb[:, kb, 128 * db:128 * db + 128],
                                     w1t[:, kb, :],
                                     start=(kb == 0), stop=(kb == 1))
                nc.scalar.mul(Wv1T[:, db, :], v_ps, 1.0 / N)

            # W2T = W2^T, W2Tn = -W2^T
            t2_ps = sps.tile([64, 16], f32, tag="t")
            nc.tensor.transpose(t2_ps, w2_sb, ident[0:16, 0:16])
            nc.scalar.copy(W2T, t2_ps)
            nc.scalar.mul(W2Tn, t2_ps, -1.0)

        if not allpairs:
            # block-diag w^T holders: zeroed once, diagonals rewritten
            wblk0 = consts.tile([128, 4, 128], f32)
            wblk1 = consts.tile([128, 4, 128], f32)
            nc.vector.memset(wblk0, 0.0)
            nc.vector.memset(wblk1, 0.0)
            wblks = [wblk0, wblk1]

        # ---------------- pools ----------------
        x_p = ctx.enter_context(tc.tile_pool(name="x", bufs=2))
        xT_p = ctx.enter_context(tc.tile_pool(name="xT", bufs=2))
        zT_p = ctx.enter_context(tc.tile_pool(name="zT", bufs=2))
        vp_p = ctx.enter_context(tc.tile_pool(name="vp", bufs=6))
        wt_p = ctx.enter_context(tc.tile_pool(name="wt", bufs=2))
        out_p = ctx.enter_context(tc.tile_pool(name="out", bufs=2))
        small_p = ctx.enter_context(tc.tile_pool(name="small", bufs=3))
        xt_ps_p = ctx.enter_context(tc.tile_pool(name="xt_ps", bufs=2, space="PSUM"))
        zt_ps_p = ctx.enter_context(tc.tile_pool(name="zt_ps", bufs=2, space="PSUM"))
        sc_ps_p = ctx.enter_context(tc.tile_pool(name="sc_ps", bufs=2, space="PSUM"))
        sub_ps_p = ctx.enter_context(tc.tile_pool(name="sub_ps", bufs=2, space="PSUM"))

        for rep in range(reps):
            for mt in range(NMT):
                r0 = mt * MT
                # ---- load x: one DMA, [128, 4(sub), 256]
                x_all = x_p.tile([128, 4, D], f32, tag="x")
                nc.sync.dma_start(
                    out=x_all,
                    in_=xin[r0:r0 + MT, :].rearrange("(s p) d -> p s d", p=128))

                # ---- transpose x -> xT [128(din), 2(blk), 512(m)] f32r
                xT = xT_p.tile([128, 2, MT], f32r, tag="xT")
                for sp in range(2):   # subtile pairs
                    tp = xt_ps_p.tile([128, 2, 2, 128], f32, tag="xtp")
                    for si in range(2):
                        for db in range(2):
                            nc.tensor.transpose(
                                tp[:, db, si, :],
                                x_all[:, 2 * sp + si, 128 * db:128 * db + 128],
                                ident)
                    nc.scalar.copy(xT[:, :, 256 * sp:256 * sp + 256], tp)

                # ---- zT [128(d'), 2(blk), 512(m)] f32 : z = x A
                zT = zT_p.tile([128, 2, MT], f32, tag="zT")
                for qb in range(2):
                    z_ps = zt_ps_p.tile([128, MT], f32, tag="zps")
                    for db in range(2):
                        nc.tensor.matmul(z_ps, A_sb[:, db, 128 * qb:128 * qb + 128],
                                         xT[:, db, :],
                                         start=(db == 0), stop=(db == 1))
                    nc.vector.tensor_copy(zT[:, qb, :], z_ps)

                outst = out_p.tile([128, 4, 48], f32, tag="outst")
                sub2s = []
                for _sp in range(2):
                    sub2t = sub_ps_p.tile([128, 512], f32, tag="sub")
                    sub2s.append(sub2t)
                vp_list = []

                # ---- V' [128(m), 64] per subtile (psum inside sub2 banks)
                for s in range(4):
                    ms = 128 * s
                    vps = sub2s[s // 2][:, 320 + 64 * (s % 2):384 + 64 * (s % 2)]
                    for db in range(2):
                        nc.tensor.matmul(vps, xT[:, db, ms:ms + 128],
                                         Wv1T[:, db, :],
                                         start=(db == 0), stop=(db == 1))
                    vp = vp_p.tile([128, 64], f32, tag="vp")
                    nc.vector.tensor_copy(vp, vps)
                    vp_list.append(vp)

                if allpairs:
                    # ---- all-pairs score [128, 4(sub), 128] + mask + softmax
                    scps = sc_ps_p.tile([128, 4, 128], f32, tag="scps")
                    for s in range(4):
                        ms = 128 * s
                        for db in range(2):
                            nc.tensor.matmul(scps[:, s, :],
                                             zT[:, db, ms:ms + 128],
                                             xT[:, db, ms:ms + 128].bitcast(f32),
                                             start=(db == 0), stop=(db == 1))
                    E_all = wt_p.tile([128, 4, 128], f32, tag="wt")
                    S4 = small_p.tile([128, 4], f32, tag="S")
                    for s in range(4):
                        nc.vector.tensor_tensor(out=scps[:, s, :],
                                                in0=scps[:, s, :], in1=mask,
                                                op=ALU.add)
                        nc.scalar.activation(out=E_all[:, s, :], in_=scps[:, s, :],
                                             func=AF.Exp, scale=1.0 / 16.0,
                                             accum_out=S4[:, s:s + 1])
                    R4 = small_p.tile([128, 4], f32, tag="R")
                    nc.vector.reciprocal(R4, S4)
                    for s in range(4):
                        nc.vector.tensor_scalar_mul(E_all[:, s, :], E_all[:, s, :],
                                                    R4[:, s:s + 1])
                    # w output: diagonal blocks, partition-aligned sbuf DMAs
                    for b4 in range(4):
                        sl = slice(32 * b4, 32 * b4 + 32)
                        nc.sync.dma_start(out=outst[sl, :, 0:32],
                                          in_=E_all[sl, :, sl])
                    # w^T with block-diag zeros = wblk, via PE transpose
                    wblk_ps = sc_ps_p.tile([128, 4, 128], f32, tag="scps")
                    for s in range(4):
                        nc.tensor.transpose(wblk_ps[:, s, :], E_all[:, s, :],
                                            ident)
                    wblk = wt_p.tile([128, 4, 128], f32, tag="wt")
                    nc.scalar.copy(wblk, wblk_ps)
                else:
                    # ---- col-tiled score + stacked softmax + DMA-built wblk
                    scps = sc_ps_p.tile([128, 4, 32], f32, tag="scps")
                    for s in range(4):
                        ms = 128 * s
                        for b4 in range(4):
                            c0 = ms + 32 * b4
                            for db in range(2):
                                nc.tensor.matmul(
                                    scps[32 * b4:32 * b4 + 32, s, :],
                                    zT[:, db, c0:c0 + 32],
                                    xT[:, db, c0:c0 + 32].bitcast(f32),
                                    start=(db == 0), stop=(db == 1),
                                    tile_position=(0, 32 * b4))
                    E4 = small_p.tile([128, 4, 32], f32, tag="E")
                    nc.scalar.activation(out=E4, in_=scps, func=AF.Exp,
                                         scale=1.0 / 16.0)
                    S4 = small_p.tile([128, 4], f32, tag="S")
                    nc.vector.tensor_reduce(out=S4, in_=E4, axis=mybir.AxisListType.X,
                                            op=ALU.add)
                    R4 = small_p.tile([128, 4], f32, tag="R")
                    nc.vector.reciprocal(R4, S4)
                    for s in range(4):
                        nc.vector.tensor_scalar_mul(outst[:, s, 0:32], E4[:, s, :],
                                                    R4[:, s:s + 1])
                    # w^T into stacked psum then one copy
                    wtall_ps = sc_ps_p.tile([32, 4, 128], f32, tag="scps")
                    for s in range(4):
                        nc.tensor.transpose(wtall_ps[0:32, s, :], outst[:, s, 0:32],
                                            ident)
                    wt_all = wt_p.tile([32, 4, 128], f32, tag="wt")
                    nc.scalar.copy(wt_all, wtall_ps)
                    wblk = wblks[mt % 2]
                    for b4 in range(4):
                        sl = slice(32 * b4, 32 * b4 + 32)
                        nc.sync.dma_start(out=wblk[sl, :, sl], in_=wt_all[:, :, sl])

                # ---- weighted + leaky relu + adv, paired subtiles
                for sp in range(2):
                    sub2 = sub2s[sp]
                    for si in range(2):
                        s = 2 * sp + si
                        wtd_ps = sub2[0:64, 128 * si:128 * si + 128]
                        nc.tensor.matmul(wtd_ps, vp_list[s], wblk[:, s, :],
                                         start=True, stop=True)
                    hTp = small_p.tile([64, 256], f32, tag="hTp")
                    nc.scalar.activation(out=hTp, in_=sub2[0:64, 0:256],
                                         func=AF.Relu)
                    hTn = small_p.tile([64, 256], f32, tag="hTn")
                    nc.vector.tensor_scalar(out=hTn, in0=sub2[0:64, 0:256],
                                            scalar1=-0.01, scalar2=0.0,
                                            op0=ALU.mult, op1=ALU.max)
                    for si in range(2):
                        s = 2 * sp + si
                        adv_ps = sub2[:, 256 + 16 * si:272 + 16 * si]
                        nc.tensor.matmul(adv_ps, hTp[:, 128 * si:128 * si + 128],
                                         W2T, start=True, stop=False)
                        nc.tensor.matmul(adv_ps, hTn[:, 128 * si:128 * si + 128],
                                         W2Tn, start=False, stop=True)
                        nc.vector.tensor_copy(outst[:, s, 32:48], adv_ps)

                # ---- one fused output DMA per m-tile
                nc.sync.dma_start(
                    out=wadv[r0:r0 + MT, :].rearrange("(s p) d -> p s d", p=128),
                    in_=outst)

    nc.compile()
    return nc


_CACHE = {}


def _get_built(reps: int = 1, allpairs: bool = True):
    key = (reps, allpairs)
    if key not in _CACHE:
        _CACHE[key] = build_kernel(reps, allpairs)
    return _CACHE[key]


def _make_mask():
    m = np.full((128, 128), -1e30, dtype=np.float32)
    for b in range(4):
        m[32 * b:32 * b + 32, 32 * b:32 * b + 32] = 0.0
    return m


def _shard_inputs(inputs):
    states = np.asarray(inputs["states"], dtype=np.float32)
    actions = np.asarray(inputs["actions"], dtype=np.float32)
    x = np.concatenate([states, actions], axis=-1).reshape(B * N, D)
    shared = {k: np.ascontiguousarray(np.asarray(inputs[k], dtype=np.float32))
              for k in ("Wk", "Wq", "Wv", "W1", "W2")}
    shared["mask"] = _make_mask()
    in_maps = []
    for c in range(N_CORES):
        m = {"xin": x[c * M:(c + 1) * M]}
        m.update(shared)
        in_maps.append(m)
    return in_maps


def kernel(**inputs):
    from concourse.bass_utils import run_bass_kernel_spmd
    nc = _get_built(1)
    in_maps = _shard_inputs(inputs)
    res = run_bass_kernel_spmd(nc, in_maps, core_ids=list(range(N_CORES)))
    return _unpack([r["wadv"] for r in res.results])


def _unpack(shards):
    wadv = np.concatenate(shards, axis=0)  # [B*N, 48]
    w = np.ascontiguousarray(wadv[:, 0:32]).reshape(B, N, N, 1)
    adv = np.ascontiguousarray(wadv[:, 32:48]).reshape(B, N, 16)
    return adv, w


if __name__ == "__main__":
    rng = np.random.default_rng(0)
    demo = {
        "states": rng.standard_normal((B, N, OBS), dtype=np.float32),
        "actions": rng.random((B, N, ACT), dtype=np.float32),
        "Wk": (rng.standard_normal((D, D), dtype=np.float32) * 0.05),
        "Wq": (rng.standard_normal((D, D), dtype=np.float32) * 0.05),
        "Wv": (rng.standard_normal((D, D), dtype=np.float32) * 0.05),
        "W1": (rng.standard_normal((64, D), dtype=np.float32) * 0.05),
        "W2": (rng.standard_normal((16, 64), dtype=np.float32) * 0.05),
    }
    adv, w = kernel(**demo)
    print("adv", adv.shape, "w", w.shape)
